# revision 5
# baseline (speedup 1.0000x reference)
"""EnergyScoreLoss Trainium2 kernel.

Math: for each element e of the [B, D] grid (flattened), with n=50 samples:
  samples_s = mean + noise_s * std,  std = sqrt(var + 1e-6)
  first   = (1/n) * sum_s |samples_s - target|
  pairsum = sum_k (2k - n + 1) * sorted(samples)_k
  energy  = first - (beta/2) * pairsum / (n(n-1)/2)
  out     = mean_e(energy)

Device formulation (per element, scale-invariant tricks):
  n'_s = noise_s / 50 (fp16),  c' = (mean - target) / std / 50 (fp16)
  first   = std * sum_s |n'_s + c'|
  pairsum = 50 * std * sum_k coef_k * sorted(n')_k          (sum coef_k = 0,
            and sorting noise == sorting samples since std > 0)
  energy  = std * (sum_s |n'_s + c'| - wsum / 49),  wsum = sum_k coef_k n'_(k)

Sharding: batch across 8 cores (65536 elements each). SBUF layout: element
e -> (partition p, col c), e = p*512 + c. Samples live in 50 blocks of 512
cols (sample-major), sorted by a pruned Batcher odd-even merge network
(403 compare-exchanges, 21 rounds) using fp16 tensor_tensor min/max at the
DVE 2x perf mode.
"""

import sys

for _p in ("/opt/trn_rl_repo", "/root/.axon_site/_ro/trn_rl_repo"):
    if _p not in sys.path:
        sys.path.insert(0, _p)

import numpy as np

N_SAMPLES = 50
N_CORES = 8
B, D = 8192, 64
V = B * D // N_CORES          # elements per core
E = V // 128                  # cols per partition
EPS = 1e-6


def _oems_rounds(n_pow2, n_real):
    """Batcher odd-even merge sort, pruned to wires < n_real.
    All comparators send min to the lower wire."""
    rounds = []
    p = 1
    while p < n_pow2:
        k = p
        while k >= 1:
            pairs = []
            for j in range(k % p, n_pow2 - k, 2 * k):
                for i in range(0, min(k, n_pow2 - j - k)):
                    a, b = i + j, i + j + k
                    if (a // (p * 2)) == (b // (p * 2)) and b < n_real:
                        pairs.append((a, b))
            if pairs:
                rounds.append(pairs)
            k //= 2
        p *= 2
    return rounds


def _runs_of(pairs):
    k = pairs[0][1] - pairs[0][0]
    lefts = sorted(a for a, _ in pairs)
    runs = []
    s = prev = lefts[0]
    for x in lefts[1:]:
        if x == prev + 1:
            prev = x
        else:
            runs.append((s, prev - s + 1))
            s = prev = x
    runs.append((s, prev - s + 1))
    return k, runs


def _group_runs(runs):
    """(start, runlen, spacing, nruns) groups with AP-regular structure."""
    groups = []
    by_len = {}
    for s, length in runs:
        by_len.setdefault(length, []).append(s)
    for length, starts in sorted(by_len.items()):
        starts.sort()
        i = 0
        while i < len(starts):
            if i + 1 < len(starts):
                d = starts[i + 1] - starts[i]
                j = i + 1
                while j + 1 < len(starts) and starts[j + 1] - starts[j] == d:
                    j += 1
                groups.append((starts[i], length, d, j - i + 1))
                i = j + 1
            else:
                groups.append((starts[i], length, 1, 1))
                i += 1
    return groups


def _build_kernel():
    import bass_rust
    import concourse.bacc as bacc
    import concourse.mybir as mybir
    import concourse.tile as tile

    f32 = mybir.dt.float32
    f16 = mybir.dt.float16
    Alu = mybir.AluOpType
    Act = mybir.ActivationFunctionType

    nc = bacc.Bacc("TRN2", target_bir_lowering=False, debug=False,
                   num_devices=N_CORES)

    noise_d = nc.declare_dram_parameter("noise", [N_SAMPLES, V], f32,
                                        isOutput=False)
    mean_d = nc.declare_dram_parameter("mean", [128, E], f32, isOutput=False)
    var_d = nc.declare_dram_parameter("variance", [128, E], f32,
                                      isOutput=False)
    target_d = nc.declare_dram_parameter("target", [128, E], f32,
                                         isOutput=False)
    out_d = nc.declare_dram_parameter("out", [1, 1], f32, isOutput=True)

    rounds = _oems_rounds(64, N_SAMPLES)

    def blk_ap(t, start, length, spacing, nruns):
        """AP over `nruns` runs of `length` consecutive blocks, run starts
        `spacing` blocks apart, starting at block `start`."""
        base = t[:]
        part_dim = list(base.ap[0])
        if nruns == 1:
            ap = [part_dim, [1, length * E]]
        else:
            ap = [part_dim, [spacing * E, nruns], [1, length * E]]
        return bass_rust.AP(tensor=base.tensor, offset=start * E, ap=ap)

    with tile.TileContext(nc) as tc:
        with (
            tc.tile_pool(name="stage", bufs=4) as stage_pool,
            tc.tile_pool(name="big", bufs=1) as big_pool,
            tc.tile_pool(name="small", bufs=1) as small_pool,
            tc.tile_pool(name="psum", bufs=1, space="PSUM") as psum_pool,
        ):
            A = big_pool.tile([128, N_SAMPLES, E], f16, tag="A")
            Bb = big_pool.tile([128, N_SAMPLES, E], f16, tag="B")

            mean_t = small_pool.tile([128, E], f32, tag="mean")
            var_t = small_pool.tile([128, E], f32, tag="var")
            target_t = small_pool.tile([128, E], f32, tag="target")
            std_t = small_pool.tile([128, E], f32, tag="std")
            rstd_t = small_pool.tile([128, E], f32, tag="rstd")
            diff_t = small_pool.tile([128, E], f32, tag="diff")
            c16_t = small_pool.tile([128, E], f16, tag="c16")
            ft_t = small_pool.tile([128, E], f32, tag="ft")
            acc_t = small_pool.tile([128, E], f32, tag="acc")
            en_t = small_pool.tile([128, E], f32, tag="en")
            part_t = small_pool.tile([128, 1], f32, tag="part")
            ones_t = small_pool.tile([128, 1], f32, tag="ones")
            eps_t = small_pool.tile([128, 1], f32, tag="eps")
            res_t = small_pool.tile([1, 1], f32, tag="res")
            ps_t = psum_pool.tile([1, 1], f32, tag="ps")

            nc.sync.dma_start(mean_t[:], mean_d[:])
            nc.sync.dma_start(var_t[:], var_d[:])
            nc.sync.dma_start(target_t[:], target_d[:])

            # std = sqrt(var + eps); rstd = 1/std
            nc.vector.memset(eps_t[:], EPS)
            nc.scalar.activation(std_t[:], var_t[:], Act.Sqrt, bias=eps_t[:])
            nc.vector.reciprocal(rstd_t[:], std_t[:])
            # c' = (mean - target) * 0.02 * rstd  -> fp16
            nc.vector.tensor_tensor(diff_t[:], mean_t[:], target_t[:],
                                    op=Alu.subtract)
            nc.vector.scalar_tensor_tensor(c16_t[:], diff_t[:], 0.02,
                                           rstd_t[:], op0=Alu.mult,
                                           op1=Alu.mult)

            # load noise rows, convert to fp16 * (1/50) into A
            for s in range(N_SAMPLES):
                st = stage_pool.tile([128, E], f32, tag="stage")
                nc.sync.dma_start(
                    st[:], noise_d[s].rearrange("(p c) -> p c", p=128))
                nc.scalar.activation(A[:, s, :], st[:], Act.Copy, scale=0.02)

            # u = n' + c' (broadcast) -> B ; first term = sum_s |u|
            c_bcast = c16_t[:].unsqueeze(1).broadcast_to((128, N_SAMPLES, E))
            nc.vector.tensor_tensor(Bb[:], A[:], c_bcast, op=Alu.add)
            nc.vector.tensor_reduce(
                ft_t[:], Bb[:].rearrange("p s c -> p c s"),
                axis=mybir.AxisListType.X, op=Alu.add,
                apply_absolute_value=True)

            # sort A (ping-pong with B)
            cur, nxt = A, Bb
            for pairs in rounds:
                k, runs = _runs_of(pairs)
                groups = _group_runs(runs)
                touched = set()
                for a, b in pairs:
                    touched.add(a)
                    touched.add(b)
                for (s0, ln, sp, nr) in groups:
                    lo_in = blk_ap(cur, s0, ln, sp, nr)
                    hi_in = blk_ap(cur, s0 + k, ln, sp, nr)
                    lo_out = blk_ap(nxt, s0, ln, sp, nr)
                    hi_out = blk_ap(nxt, s0 + k, ln, sp, nr)
                    nc.vector.tensor_tensor(lo_out, lo_in, hi_in, op=Alu.min)
                    nc.vector.tensor_tensor(hi_out, lo_in, hi_in, op=Alu.max)
                # copy untouched wires
                unt = sorted(set(range(N_SAMPLES)) - touched)
                if unt:
                    cruns = []
                    s = prev = unt[0]
                    for x in unt[1:]:
                        if x == prev + 1:
                            prev = x
                        else:
                            cruns.append((s, prev - s + 1))
                            s = prev = x
                    cruns.append((s, prev - s + 1))
                    for (cs, cl) in cruns:
                        nc.vector.tensor_copy(blk_ap(nxt, cs, cl, 1, 1),
                                              blk_ap(cur, cs, cl, 1, 1))
                cur, nxt = nxt, cur

            # weighted sum: d_j = m_{25+j} - m_{24-j}, acc = sum_j coef_j d_j
            half = N_SAMPLES // 2
            hi_half = cur[:][:, half:N_SAMPLES, :]
            lo_half = cur[:][:, half - 1::-1, :]
            d_t = nxt  # reuse as scratch: first `half` blocks, fp32 would not
            # fit; keep fp16 diffs then accumulate in fp32 via STT
            nc.vector.tensor_tensor(d_t[:][:, 0:half, :], hi_half, lo_half,
                                    op=Alu.subtract)
            nc.vector.memset(acc_t[:], 0.0)
            for j in range(half):
                coef = float(2 * (half + j) - (N_SAMPLES - 1))
                nc.vector.scalar_tensor_tensor(
                    acc_t[:], d_t[:][:, j, :], coef, acc_t[:],
                    op0=Alu.mult, op1=Alu.add)

            # energy = std * (ft - acc/49); partial = sum(energy)
            nc.vector.scalar_tensor_tensor(en_t[:], acc_t[:],
                                           -1.0 / (N_SAMPLES - 1), ft_t[:],
                                           op0=Alu.mult, op1=Alu.add)
            nc.vector.tensor_tensor(en_t[:], en_t[:], std_t[:], op=Alu.mult)
            nc.vector.tensor_reduce(part_t[:], en_t[:],
                                    axis=mybir.AxisListType.X, op=Alu.add)
            nc.vector.memset(ones_t[:], 1.0)
            nc.tensor.matmul(ps_t[:], part_t[:], ones_t[:])
            nc.scalar.copy(res_t[:], ps_t[:])
            nc.sync.dma_start(out_d[:], res_t[:])

    nc.compile()
    return nc


_NC_CACHE = None


def _get_nc():
    global _NC_CACHE
    if _NC_CACHE is None:
        _NC_CACHE = _build_kernel()
    return _NC_CACHE


def kernel(mean, variance, noise, target):
    from concourse.bass_utils import run_bass_kernel_spmd

    nc = _get_nc()

    mean = np.ascontiguousarray(mean, dtype=np.float32).reshape(B * D)
    variance = np.ascontiguousarray(variance, dtype=np.float32).reshape(B * D)
    target = np.ascontiguousarray(target, dtype=np.float32).reshape(B * D)
    noise = np.ascontiguousarray(noise, dtype=np.float32).reshape(N_SAMPLES,
                                                                  B * D)

    in_maps = []
    for c in range(N_CORES):
        sl = slice(c * V, (c + 1) * V)
        in_maps.append({
            "noise": np.ascontiguousarray(noise[:, sl]),
            "mean": mean[sl].reshape(128, E),
            "variance": variance[sl].reshape(128, E),
            "target": target[sl].reshape(128, E),
        })

    res = run_bass_kernel_spmd(nc, in_maps, core_ids=list(range(N_CORES)))
    total = sum(float(res.results[c]["out"][0, 0]) for c in range(N_CORES))
    return np.float32(total / (B * D))


# revision 10
# speedup vs baseline: 1.0154x; 1.0154x over previous
"""EnergyScoreLoss Trainium2 kernel.

Math: for each element e of the [B, D] grid (flattened), with n=50 samples:
  samples_s = mean + noise_s * std,  std = sqrt(var + 1e-6)
  first   = (1/n) * sum_s |samples_s - target|
  pairsum = sum_k (2k - n + 1) * sorted(samples)_k
  energy  = first - (beta/2) * pairsum / (n(n-1)/2)
  out     = mean_e(energy)

Device formulation (per element, scale/shift-invariant tricks):
  u_s = noise_s/50 + c',  c' = (mean - target) / std / 50   (fp16)
  first   = std * sum_s |u_s|
  sorting u == sorting samples (std > 0), and since sum coef_k = 0 the
  shift by c' drops out of the weighted sum:
  energy  = std * (sum_s |u_s| - wsum / 49),  wsum = sum_k coef_k u_(k)

Sharding: batch across 8 cores (65536 elements each). SBUF layout: element
e -> (partition p, col c), e = p*512 + c. Samples live in 50 blocks of 512
cols (sample-major), sorted by a pruned Batcher odd-even merge network
(403 compare-exchanges, 21 rounds) using fp16 tensor_tensor min/max at the
DVE 2x perf mode. First term: GPSIMD abs + DVE fp16 pairwise tree-sum.
"""

import sys

for _p in ("/opt/trn_rl_repo", "/root/.axon_site/_ro/trn_rl_repo"):
    if _p not in sys.path:
        sys.path.insert(0, _p)

import numpy as np

N_SAMPLES = 50
N_CORES = 8
B, D = 8192, 64
V = B * D // N_CORES          # elements per core
E = V // 128                  # cols per partition
EPS = 1e-6


def _oems_rounds(n_pow2, n_real):
    """Batcher odd-even merge sort, pruned to wires < n_real.
    All comparators send min to the lower wire."""
    rounds = []
    p = 1
    while p < n_pow2:
        k = p
        while k >= 1:
            pairs = []
            for j in range(k % p, n_pow2 - k, 2 * k):
                for i in range(0, min(k, n_pow2 - j - k)):
                    a, b = i + j, i + j + k
                    if (a // (p * 2)) == (b // (p * 2)) and b < n_real:
                        pairs.append((a, b))
            if pairs:
                rounds.append(pairs)
            k //= 2
        p *= 2
    return rounds


def _runs_of(pairs):
    k = pairs[0][1] - pairs[0][0]
    lefts = sorted(a for a, _ in pairs)
    runs = []
    s = prev = lefts[0]
    for x in lefts[1:]:
        if x == prev + 1:
            prev = x
        else:
            runs.append((s, prev - s + 1))
            s = prev = x
    runs.append((s, prev - s + 1))
    return k, runs


def _group_runs(runs):
    """(start, runlen, spacing, nruns) groups with AP-regular structure."""
    groups = []
    by_len = {}
    for s, length in runs:
        by_len.setdefault(length, []).append(s)
    for length, starts in sorted(by_len.items()):
        starts.sort()
        i = 0
        while i < len(starts):
            if i + 1 < len(starts):
                d = starts[i + 1] - starts[i]
                j = i + 1
                while j + 1 < len(starts) and starts[j + 1] - starts[j] == d:
                    j += 1
                groups.append((starts[i], length, d, j - i + 1))
                i = j + 1
            else:
                groups.append((starts[i], length, 1, 1))
                i += 1
    return groups


def _wire_runs(wires):
    runs = []
    if not wires:
        return runs
    s = prev = wires[0]
    for x in wires[1:]:
        if x == prev + 1:
            prev = x
        else:
            runs.append((s, prev - s + 1))
            s = prev = x
    runs.append((s, prev - s + 1))
    return runs


def _build_kernel():
    import bass_rust
    import concourse.bacc as bacc
    import concourse.mybir as mybir
    import concourse.tile as tile

    f32 = mybir.dt.float32
    f16 = mybir.dt.float16
    Alu = mybir.AluOpType
    Act = mybir.ActivationFunctionType

    nc = bacc.Bacc("TRN2", target_bir_lowering=False, debug=False,
                   num_devices=N_CORES)

    noise_d = nc.declare_dram_parameter("noise", [N_SAMPLES, V], f32,
                                        isOutput=False)
    mean_d = nc.declare_dram_parameter("mean", [128, E], f32, isOutput=False)
    var_d = nc.declare_dram_parameter("variance", [128, E], f32,
                                      isOutput=False)
    target_d = nc.declare_dram_parameter("target", [128, E], f32,
                                         isOutput=False)
    out_d = nc.declare_dram_parameter("out", [1, 1], f32, isOutput=True)

    rounds = _oems_rounds(64, N_SAMPLES)

    def blk_ap(t, start, length, spacing, nruns):
        """AP over `nruns` runs of `length` consecutive blocks, run starts
        `spacing` blocks apart, starting at block `start`."""
        base = t[:]
        part_dim = list(base.ap[0])
        if nruns == 1:
            ap = [part_dim, [1, length * E]]
        else:
            ap = [part_dim, [spacing * E, nruns], [1, length * E]]
        return bass_rust.AP(tensor=base.tensor, offset=start * E, ap=ap)

    with tile.TileContext(nc) as tc:
        with (
            tc.tile_pool(name="stage", bufs=18) as stage_pool,
            tc.tile_pool(name="big", bufs=1) as big_pool,
            tc.tile_pool(name="small", bufs=1) as small_pool,
            tc.tile_pool(name="psum", bufs=1, space="PSUM") as psum_pool,
        ):
            U = big_pool.tile([128, N_SAMPLES, E], f16, tag="U")
            W = big_pool.tile([128, N_SAMPLES, E], f16, tag="W")

            mean_t = small_pool.tile([128, E], f32, tag="mean")
            var_t = small_pool.tile([128, E], f32, tag="var")
            target_t = small_pool.tile([128, E], f32, tag="target")
            std_t = small_pool.tile([128, E], f32, tag="std")
            rstd_t = small_pool.tile([128, E], f32, tag="rstd")
            diff_t = small_pool.tile([128, E], f32, tag="diff")
            c16_t = small_pool.tile([128, E], f16, tag="c16")
            ft_t = small_pool.tile([128, E], f32, tag="ft")
            acc_t = small_pool.tile([128, E], f32, tag="acc")
            en_t = small_pool.tile([128, E], f32, tag="en")
            part_t = small_pool.tile([128, 1], f32, tag="part")
            ones_t = small_pool.tile([128, 1], f32, tag="ones")
            eps_t = small_pool.tile([128, 1], f32, tag="eps")
            res_t = small_pool.tile([1, 1], f32, tag="res")
            ps_t = psum_pool.tile([1, 1], f32, tag="ps")

            nc.sync.dma_start(mean_t[:], mean_d[:])
            nc.sync.dma_start(var_t[:], var_d[:])
            nc.sync.dma_start(target_t[:], target_d[:])

            # std = sqrt(var + eps); rstd = 1/std
            nc.vector.memset(eps_t[:], EPS)
            nc.scalar.activation(std_t[:], var_t[:], Act.Sqrt, bias=eps_t[:])
            nc.vector.reciprocal(rstd_t[:], std_t[:])
            # c' = (mean - target) * 0.02 * rstd  -> fp16
            nc.vector.tensor_tensor(diff_t[:], mean_t[:], target_t[:],
                                    op=Alu.subtract)
            nc.vector.scalar_tensor_tensor(c16_t[:], diff_t[:], 0.02,
                                           rstd_t[:], op0=Alu.mult,
                                           op1=Alu.mult)

            # input pipeline per sample block s:
            #   DMA fp32 row -> stage ; ACT: W_s = stage * 0.02 (fp16)
            #   DVE: U_s = W_s + c'
            for s in range(N_SAMPLES):
                st = stage_pool.tile([128, E], f32, tag="stage")
                nc.sync.dma_start(
                    st[:], noise_d[s].rearrange("(p c) -> p c", p=128))
                nc.scalar.activation(W[:, s, :], st[:], Act.Copy, scale=0.02)
                nc.vector.tensor_tensor(U[:, s, :], W[:, s, :], c16_t[:],
                                        op=Alu.add)
                # W_s = |U_s| via max(u, -u)
                nc.vector.tensor_scalar_mul(W[:, s, :], U[:, s, :], -1.0)
                nc.vector.tensor_tensor(W[:, s, :], U[:, s, :], W[:, s, :],
                                        op=Alu.max)

            # first term: pairwise tree-sum of |u| blocks (in place in W).
            # level pairing (i, i+half) keeps writes behind reads.
            cnt = N_SAMPLES
            while cnt > 1:
                half = cnt // 2
                odd = cnt % 2
                # W[j] += W[j + half + odd] for j in [odd, half+odd)
                lo = blk_ap(W, odd, half, 1, 1)
                hi = blk_ap(W, half + odd, half, 1, 1)
                if cnt == 2:
                    # final add -> fp32 ft
                    nc.vector.tensor_tensor(ft_t[:], lo, hi, op=Alu.add)
                else:
                    nc.vector.tensor_tensor(lo, lo, hi, op=Alu.add)
                cnt = half + odd

            # sort U (ping-pong with W); untouched-wire copies go to the
            # otherwise-idle Scalar engine so the DVE only does min/max
            cur, oth = U, W
            for pairs in rounds:
                k, runs = _runs_of(pairs)
                groups = _group_runs(runs)
                touched = set()
                for a, b in pairs:
                    touched.add(a)
                    touched.add(b)
                for (s0, ln, sp, nr) in groups:
                    lo_in = blk_ap(cur, s0, ln, sp, nr)
                    hi_in = blk_ap(cur, s0 + k, ln, sp, nr)
                    lo_out = blk_ap(oth, s0, ln, sp, nr)
                    hi_out = blk_ap(oth, s0 + k, ln, sp, nr)
                    nc.vector.tensor_tensor(lo_out, lo_in, hi_in, op=Alu.min)
                    nc.vector.tensor_tensor(hi_out, lo_in, hi_in, op=Alu.max)
                unt = sorted(set(range(N_SAMPLES)) - touched)
                for (cs, cl) in _wire_runs(unt):
                    nc.scalar.copy(blk_ap(oth, cs, cl, 1, 1),
                                   blk_ap(cur, cs, cl, 1, 1))
                cur, oth = oth, cur

            # weighted sum: d_j = m_{25+j} - m_{24-j}, acc = sum_j coef_j d_j
            half = N_SAMPLES // 2
            hi_half = cur[:][:, half:N_SAMPLES, :]
            lo_half = cur[:][:, half - 1::-1, :]
            nc.vector.tensor_tensor(oth[:][:, 0:half, :], hi_half, lo_half,
                                    op=Alu.subtract)
            nc.vector.memset(acc_t[:], 0.0)
            for j in range(half):
                coef = float(2 * (half + j) - (N_SAMPLES - 1))
                nc.vector.scalar_tensor_tensor(
                    acc_t[:], oth[:][:, j, :], coef, acc_t[:],
                    op0=Alu.mult, op1=Alu.add)

            # energy = std * (ft - acc/49); partial = sum(energy)
            nc.vector.scalar_tensor_tensor(en_t[:], acc_t[:],
                                           -1.0 / (N_SAMPLES - 1), ft_t[:],
                                           op0=Alu.mult, op1=Alu.add)
            nc.vector.tensor_tensor(en_t[:], en_t[:], std_t[:], op=Alu.mult)
            nc.vector.tensor_reduce(part_t[:], en_t[:],
                                    axis=mybir.AxisListType.X, op=Alu.add)
            nc.vector.memset(ones_t[:], 1.0)
            nc.tensor.matmul(ps_t[:], part_t[:], ones_t[:])
            nc.scalar.copy(res_t[:], ps_t[:])
            nc.sync.dma_start(out_d[:], res_t[:])

    nc.compile()
    return nc


_NC_CACHE = None


def _get_nc():
    global _NC_CACHE
    if _NC_CACHE is None:
        _NC_CACHE = _build_kernel()
    return _NC_CACHE


def kernel(mean, variance, noise, target):
    from concourse.bass_utils import run_bass_kernel_spmd

    nc = _get_nc()

    mean = np.ascontiguousarray(mean, dtype=np.float32).reshape(B * D)
    variance = np.ascontiguousarray(variance, dtype=np.float32).reshape(B * D)
    target = np.ascontiguousarray(target, dtype=np.float32).reshape(B * D)
    noise = np.ascontiguousarray(noise, dtype=np.float32).reshape(N_SAMPLES,
                                                                  B * D)

    in_maps = []
    for c in range(N_CORES):
        sl = slice(c * V, (c + 1) * V)
        in_maps.append({
            "noise": np.ascontiguousarray(noise[:, sl]),
            "mean": mean[sl].reshape(128, E),
            "variance": variance[sl].reshape(128, E),
            "target": target[sl].reshape(128, E),
        })

    res = run_bass_kernel_spmd(nc, in_maps, core_ids=list(range(N_CORES)))
    total = sum(float(res.results[c]["out"][0, 0]) for c in range(N_CORES))
    return np.float32(total / (B * D))


# revision 13
# speedup vs baseline: 1.2173x; 1.1988x over previous
"""EnergyScoreLoss Trainium2 kernel.

Math: for each element e of the [B, D] grid (flattened), with n=50 samples:
  samples_s = mean + noise_s * std,  std = sqrt(var + 1e-6)
  first   = (1/n) * sum_s |samples_s - target|
  pairsum = sum_k (2k - n + 1) * sorted(samples)_k
  energy  = first - (beta/2) * pairsum / (n(n-1)/2)
  out     = mean_e(energy)

Device formulation (per element, scale/shift-invariant tricks):
  u_s = noise_s/50 + c',  c' = (mean - target) / std / 50   (fp16)
  first   = std * sum_s |u_s|
  sorting u == sorting samples (std > 0), and since sum coef_k = 0 the
  shift by c' drops out of the weighted sum:
  energy  = std * (sum_s |u_s| - wsum / 49),  wsum = sum_k coef_k u_(k)

Sharding: batch across 8 cores (65536 elements each). SBUF layout: element
e -> (partition p, col c), e = p*512 + c. Samples live in 50 blocks of 512
cols (sample-major), sorted by a pruned Batcher odd-even merge network
(403 compare-exchanges, 21 rounds) using fp16 tensor_tensor min/max at the
DVE 2x perf mode. First term: GPSIMD abs + DVE fp16 pairwise tree-sum.
"""

import sys

for _p in ("/opt/trn_rl_repo", "/root/.axon_site/_ro/trn_rl_repo"):
    if _p not in sys.path:
        sys.path.insert(0, _p)

import numpy as np

N_SAMPLES = 50
N_CORES = 8
B, D = 8192, 64
V = B * D // N_CORES          # elements per core
E = V // 128                  # cols per partition
EPS = 1e-6


def _oems_rounds(n_pow2, n_real):
    """Batcher odd-even merge sort, pruned to wires < n_real.
    All comparators send min to the lower wire."""
    rounds = []
    p = 1
    while p < n_pow2:
        k = p
        while k >= 1:
            pairs = []
            for j in range(k % p, n_pow2 - k, 2 * k):
                for i in range(0, min(k, n_pow2 - j - k)):
                    a, b = i + j, i + j + k
                    if (a // (p * 2)) == (b // (p * 2)) and b < n_real:
                        pairs.append((a, b))
            if pairs:
                rounds.append(pairs)
            k //= 2
        p *= 2
    return rounds


def _runs_of(pairs):
    k = pairs[0][1] - pairs[0][0]
    lefts = sorted(a for a, _ in pairs)
    runs = []
    s = prev = lefts[0]
    for x in lefts[1:]:
        if x == prev + 1:
            prev = x
        else:
            runs.append((s, prev - s + 1))
            s = prev = x
    runs.append((s, prev - s + 1))
    return k, runs


def _group_runs(runs):
    """(start, runlen, spacing, nruns) groups with AP-regular structure."""
    groups = []
    by_len = {}
    for s, length in runs:
        by_len.setdefault(length, []).append(s)
    for length, starts in sorted(by_len.items()):
        starts.sort()
        i = 0
        while i < len(starts):
            if i + 1 < len(starts):
                d = starts[i + 1] - starts[i]
                j = i + 1
                while j + 1 < len(starts) and starts[j + 1] - starts[j] == d:
                    j += 1
                groups.append((starts[i], length, d, j - i + 1))
                i = j + 1
            else:
                groups.append((starts[i], length, 1, 1))
                i += 1
    return groups


def _wire_runs(wires):
    runs = []
    if not wires:
        return runs
    s = prev = wires[0]
    for x in wires[1:]:
        if x == prev + 1:
            prev = x
        else:
            runs.append((s, prev - s + 1))
            s = prev = x
    runs.append((s, prev - s + 1))
    return runs


def _build_kernel():
    import bass_rust
    import concourse.bacc as bacc
    import concourse.mybir as mybir
    import concourse.tile as tile

    f32 = mybir.dt.float32
    f16 = mybir.dt.float16
    Alu = mybir.AluOpType
    Act = mybir.ActivationFunctionType

    nc = bacc.Bacc("TRN2", target_bir_lowering=False, debug=False,
                   num_devices=N_CORES)

    noise_d = nc.declare_dram_parameter("noise", [N_SAMPLES, V], f32,
                                        isOutput=False)
    mean_d = nc.declare_dram_parameter("mean", [128, E], f32, isOutput=False)
    var_d = nc.declare_dram_parameter("variance", [128, E], f32,
                                      isOutput=False)
    target_d = nc.declare_dram_parameter("target", [128, E], f32,
                                         isOutput=False)
    out_d = nc.declare_dram_parameter("out", [1, 1], f32, isOutput=True)

    rounds = _oems_rounds(64, N_SAMPLES)

    def blk_ap(t, start, length, spacing, nruns):
        """AP over `nruns` runs of `length` consecutive blocks, run starts
        `spacing` blocks apart, starting at block `start`."""
        base = t[:]
        part_dim = list(base.ap[0])
        if nruns == 1:
            ap = [part_dim, [1, length * E]]
        else:
            ap = [part_dim, [spacing * E, nruns], [1, length * E]]
        return bass_rust.AP(tensor=base.tensor, offset=start * E, ap=ap)

    with tile.TileContext(nc) as tc:
        with (
            tc.tile_pool(name="stage", bufs=18) as stage_pool,
            tc.tile_pool(name="big", bufs=1) as big_pool,
            tc.tile_pool(name="small", bufs=1) as small_pool,
            tc.tile_pool(name="psum", bufs=1, space="PSUM") as psum_pool,
        ):
            U = big_pool.tile([128, N_SAMPLES, E], f16, tag="U")
            W = big_pool.tile([128, N_SAMPLES, E], f16, tag="W")

            mean_t = small_pool.tile([128, E], f32, tag="mean")
            var_t = small_pool.tile([128, E], f32, tag="var")
            target_t = small_pool.tile([128, E], f32, tag="target")
            std_t = small_pool.tile([128, E], f32, tag="std")
            rstd_t = small_pool.tile([128, E], f32, tag="rstd")
            diff_t = small_pool.tile([128, E], f32, tag="diff")
            c16_t = small_pool.tile([128, E], f16, tag="c16")
            ft_t = small_pool.tile([128, E], f32, tag="ft")
            acc_t = small_pool.tile([128, E], f32, tag="acc")
            en_t = small_pool.tile([128, E], f32, tag="en")
            part_t = small_pool.tile([128, 1], f32, tag="part")
            ones_t = small_pool.tile([128, 1], f32, tag="ones")
            eps_t = small_pool.tile([128, 1], f32, tag="eps")
            res_t = small_pool.tile([1, 1], f32, tag="res")
            ps_t = psum_pool.tile([1, 1], f32, tag="ps")

            nc.sync.dma_start(mean_t[:], mean_d[:])
            nc.sync.dma_start(var_t[:], var_d[:])
            nc.sync.dma_start(target_t[:], target_d[:])

            # std = sqrt(var + eps); rstd = 1/std
            nc.vector.memset(eps_t[:], EPS)
            nc.scalar.activation(std_t[:], var_t[:], Act.Sqrt, bias=eps_t[:])
            nc.vector.reciprocal(rstd_t[:], std_t[:])
            # c' = (mean - target) * 0.02 * rstd  -> fp16
            nc.vector.tensor_tensor(diff_t[:], mean_t[:], target_t[:],
                                    op=Alu.subtract)
            nc.vector.scalar_tensor_tensor(c16_t[:], diff_t[:], 0.02,
                                           rstd_t[:], op0=Alu.mult,
                                           op1=Alu.mult)

            # input pipeline per sample block s:
            #   DMA fp32 row -> stage ; ACT: W_s = stage * 0.02 (fp16)
            #   DVE: U_s = W_s + c'
            for s in range(N_SAMPLES):
                st = stage_pool.tile([128, E], f32, tag="stage")
                nc.sync.dma_start(
                    st[:], noise_d[s].rearrange("(p c) -> p c", p=128))
                nc.scalar.activation(W[:, s, :], st[:], Act.Copy, scale=0.02)
                nc.vector.tensor_tensor(U[:, s, :], W[:, s, :], c16_t[:],
                                        op=Alu.add)
                # W_s = relu(U_s)  (4x-mode tensor_scalar)
                nc.vector.tensor_scalar_max(W[:, s, :], U[:, s, :], 0.0)

            # first term: sum_s |u_s| = 2*sum relu(u) - sum u.
            # Tree-sum relu blocks in place in W; tree-sum U blocks into the
            # W blocks freed by the first relu-tree level.
            def tree_sum(src, base0, out32, lvl1_dst=None):
                """Pairwise tree over 50 blocks of `src` starting at block
                base0 (in place), final fp32 add into out32. If lvl1_dst is
                (tile, blockoff), level-1 results go there instead."""
                t, off, cnt = src, base0, N_SAMPLES
                while cnt > 1:
                    half = cnt // 2
                    odd = cnt % 2
                    lo = blk_ap(t, off + odd, half, 1, 1)
                    hi = blk_ap(t, off + half + odd, half, 1, 1)
                    if cnt == 2:
                        nc.vector.tensor_tensor(out32[:], lo, hi, op=Alu.add)
                    elif lvl1_dst is not None:
                        dt, doff = lvl1_dst
                        dst = blk_ap(dt, doff + odd, half, 1, 1)
                        if odd:
                            nc.scalar.copy(blk_ap(dt, doff, 1, 1, 1),
                                           blk_ap(t, off, 1, 1, 1))
                        nc.vector.tensor_tensor(dst, lo, hi, op=Alu.add)
                        t, off = dt, doff
                        lvl1_dst = None
                    else:
                        nc.vector.tensor_tensor(lo, lo, hi, op=Alu.add)
                    cnt = half + odd

            relu_sum = small_pool.tile([128, E], f32, tag="relu_sum")
            usum_t = small_pool.tile([128, E], f32, tag="usum")
            tree_sum(W, 0, relu_sum)               # consumes W[25..50) first
            tree_sum(U, 0, usum_t, lvl1_dst=(W, 25))
            # ft = 2*relu_sum - usum
            nc.vector.scalar_tensor_tensor(ft_t[:], relu_sum[:], 2.0,
                                           usum_t[:], op0=Alu.mult,
                                           op1=Alu.subtract)

            # sort U (ping-pong with W); untouched-wire copies go to the
            # otherwise-idle Scalar engine so the DVE only does min/max
            cur, oth = U, W
            for pairs in rounds:
                k, runs = _runs_of(pairs)
                groups = _group_runs(runs)
                touched = set()
                for a, b in pairs:
                    touched.add(a)
                    touched.add(b)
                for (s0, ln, sp, nr) in groups:
                    lo_in = blk_ap(cur, s0, ln, sp, nr)
                    hi_in = blk_ap(cur, s0 + k, ln, sp, nr)
                    lo_out = blk_ap(oth, s0, ln, sp, nr)
                    hi_out = blk_ap(oth, s0 + k, ln, sp, nr)
                    nc.vector.tensor_tensor(lo_out, lo_in, hi_in, op=Alu.min)
                    nc.vector.tensor_tensor(hi_out, lo_in, hi_in, op=Alu.max)
                unt = sorted(set(range(N_SAMPLES)) - touched)
                for (cs, cl) in _wire_runs(unt):
                    nc.sync.dma_start(blk_ap(oth, cs, cl, 1, 1),
                                      blk_ap(cur, cs, cl, 1, 1))
                cur, oth = oth, cur

            # weighted sum: d_j = m_{25+j} - m_{24-j}, acc = sum_j coef_j d_j
            half = N_SAMPLES // 2
            hi_half = cur[:][:, half:N_SAMPLES, :]
            lo_half = cur[:][:, half - 1::-1, :]
            nc.vector.tensor_tensor(oth[:][:, 0:half, :], hi_half, lo_half,
                                    op=Alu.subtract)
            acc16_t = small_pool.tile([128, E], f16, tag="acc16")
            nc.vector.memset(acc16_t[:], 0.0)
            for j in range(half):
                coef = float(2 * (half + j) - (N_SAMPLES - 1))
                nc.vector.scalar_tensor_tensor(
                    acc16_t[:], oth[:][:, j, :], coef, acc16_t[:],
                    op0=Alu.mult, op1=Alu.add)

            # energy = std * (ft - acc/49); partial = sum(energy)
            nc.vector.scalar_tensor_tensor(en_t[:], acc16_t[:],
                                           -1.0 / (N_SAMPLES - 1), ft_t[:],
                                           op0=Alu.mult, op1=Alu.add)
            nc.vector.tensor_tensor(en_t[:], en_t[:], std_t[:], op=Alu.mult)
            nc.vector.tensor_reduce(part_t[:], en_t[:],
                                    axis=mybir.AxisListType.X, op=Alu.add)
            nc.vector.memset(ones_t[:], 1.0)
            nc.tensor.matmul(ps_t[:], part_t[:], ones_t[:])
            nc.scalar.copy(res_t[:], ps_t[:])
            nc.sync.dma_start(out_d[:], res_t[:])

    nc.compile()
    return nc


_NC_CACHE = None


def _get_nc():
    global _NC_CACHE
    if _NC_CACHE is None:
        _NC_CACHE = _build_kernel()
    return _NC_CACHE


def kernel(mean, variance, noise, target):
    from concourse.bass_utils import run_bass_kernel_spmd

    nc = _get_nc()

    mean = np.ascontiguousarray(mean, dtype=np.float32).reshape(B * D)
    variance = np.ascontiguousarray(variance, dtype=np.float32).reshape(B * D)
    target = np.ascontiguousarray(target, dtype=np.float32).reshape(B * D)
    noise = np.ascontiguousarray(noise, dtype=np.float32).reshape(N_SAMPLES,
                                                                  B * D)

    in_maps = []
    for c in range(N_CORES):
        sl = slice(c * V, (c + 1) * V)
        in_maps.append({
            "noise": np.ascontiguousarray(noise[:, sl]),
            "mean": mean[sl].reshape(128, E),
            "variance": variance[sl].reshape(128, E),
            "target": target[sl].reshape(128, E),
        })

    res = run_bass_kernel_spmd(nc, in_maps, core_ids=list(range(N_CORES)))
    total = sum(float(res.results[c]["out"][0, 0]) for c in range(N_CORES))
    return np.float32(total / (B * D))
